# revision 9
# baseline (speedup 1.0000x reference)
"""Trainium2 Bass kernel for the ConvFeatureExtractor problem.

Reference computation (all f32):
    matches[f, i] = sum_j kmer_params[f, kmer_idcs[i, j], j]      # (F, M)
    probs = softmax(matches / temperature, axis=1)                # over M
    pooled = freq @ probs.T                                       # (B, F)
    profile = pooled / pooled.sum(axis=1, keepdims=True)

Shapes: B=1024, M=4096 (=4^6 kmers), F=8192 filters, K=6, 4 bases.

Kernel strategy (8 NeuronCores, filter-sharded: FL = F/8 = 1024 per core):
  * Host folds 1/T, the per-(filter,position) max-shift and a x128 scale
    into params_eff; softmax is invariant to the shift, and the scale
    cancels in the final normalization.  E' = exp(matches_eff) lands in
    (0, 128] which fits fp8e4 (max 240) with all mass in normal range.
  * matches^T via K=24 matmuls, 4-row-packed into the PE array
    (tile_position row groups), exp on ScalarE written as fp8 E.
  * Z[f] = sum_i E[i, f] via DoubleRow ones-matmuls interleaved with
    phase A (broadcast across partitions for free).
  * U = freq @ E^T as fp8 DoubleRow matmuls (2 MACs/cell/cycle).
  * One fused DVE tensor_tensor_reduce per batch tile applies 1/Z and
    produces the per-row sums s in the same pass.
  * s AllReduce over the 8 cores split in two chunks so the first chunk's
    collective latency hides under the tail of the main GEMM.
Each core returns its (B, FL) f32 slice; host concatenates along F.
"""

import os

import numpy as np
import ml_dtypes

import concourse.bass as bass  # noqa: F401
import concourse.tile as tile
from concourse import bacc, mybir
from concourse.bass_utils import run_bass_kernel_spmd

NCORES = 8
B = 1024           # batch
M = 4096           # 4^6 kmers
F = 8192           # filters
KMER = 6           # kmer length
NBASE = 4
KK = NBASE * KMER  # 24 flattened (base, position)
FL = F // NCORES   # 1024 filters per core

MT = M // 128      # 32 contraction subtiles of 128
KP = MT // 2       # 16 DoubleRow k-pairs
NBT = B // 128     # 8 batch tiles
SPLIT_BT = 6       # batch tiles covered by the first (overlapped) AllReduce

BF16 = mybir.dt.bfloat16
FP8 = mybir.dt.float8e4
F32 = mybir.dt.float32
AFT = mybir.ActivationFunctionType
ALU = mybir.AluOpType
DR = mybir.MatmulPerfMode.DoubleRow

USE_FP8 = os.environ.get("KERNEL_BF16", "") in ("", "0")

_CACHE: dict = {}


def _body(tc, freqT, oh4, par4, out):
    nc = tc.nc
    stage = os.environ.get("KERNEL_STAGE", "")
    edt = FP8 if USE_FP8 else BF16
    with (
        tc.tile_pool(name="res", bufs=1) as res,
        tc.tile_pool(name="dram", bufs=1, space="DRAM") as dram,
    ):
        # ---------- constants / small inputs ----------
        oh_sb = res.tile([128, 8 * 128], BF16)      # 4-row-packed onehot^T
        nc.sync.dma_start(oh_sb[:], oh4[:])
        par_sb = res.tile([128, FL], BF16)          # 4-row-packed params_eff^T
        nc.sync.dma_start(par_sb[:], par4[:])
        ones_e = res.tile([128, 2, 128], edt)       # DoubleRow ones lhsT
        nc.vector.memset(ones_e[:], 1.0)
        # memset-initialized rhs for PE warm-up matmuls (no DMA dependency)
        warm_rhs = res.tile([128, 2, 512], edt)
        nc.vector.memset(warm_rhs[:], 1.0)

        # ---------- stream in freq^T (fp8/bf16, k-subtile major) ----------
        freq_sb = res.tile([128, MT, B], edt)
        for k in range(MT):
            nc.sync.dma_start(freq_sb[:, k, :], freqT[k * 128:(k + 1) * 128, :])

        E_sb = res.tile([128, MT, FL], edt)
        invz_bc = res.tile([128, FL], F32)

        # ---------- phase A: matches -> E = exp(.); Z via ones-matmuls ----
        with tc.tile_pool(name="psA", bufs=1, space="PSUM") as psA:
            zz = psA.tile([128, FL], F32, tag="zz")

            # ---- PE warm-up: dep-free matmuls fill the input-DMA head so
            # the HAM clock gate reaches 8/8 before real compute starts.
            warm_ps = psA.tile([128, 512], F32, tag="warm")
            for i in range(24):
                if USE_FP8:
                    nc.tensor.matmul(warm_ps[:], lhsT=ones_e[:],
                                     rhs=warm_rhs[:], start=True, stop=True,
                                     perf_mode=DR)
                else:
                    nc.tensor.matmul(warm_ps[:], lhsT=ones_e[:, 0, :],
                                     rhs=warm_rhs[:, 0, :],
                                     start=True, stop=True)

            def z_mms(j, fcs):
                # Z accumulation for the pack produced in iteration j
                for kp, fc in ((kpj, fcj) for kpj in (2 * j, 2 * j + 1)
                               for fcj in fcs):
                    if USE_FP8:
                        nc.tensor.matmul(
                            zz[:, fc * 512:(fc + 1) * 512],
                            lhsT=ones_e[:],
                            rhs=E_sb[:, 2 * kp:2 * kp + 2,
                                     fc * 512:(fc + 1) * 512],
                            start=(kp == 0), stop=(kp == KP - 1),
                            perf_mode=DR)
                    else:
                        for k in (2 * kp, 2 * kp + 1):
                            nc.tensor.matmul(
                                zz[:, fc * 512:(fc + 1) * 512],
                                lhsT=ones_e[:, 0, :],
                                rhs=E_sb[:, k, fc * 512:(fc + 1) * 512],
                                start=(k == 0), stop=(k == MT - 1))

            for j in range(8):          # packs of 4 m-tiles (t = 4j+g)
                for fc in range(2):
                    pmA = psA.tile([128, 2, 512], F32, tag="pm", bufs=2,
                                   name=f"pmA{j}_{fc}")
                    pmB = psA.tile([128, 2, 512], F32, tag="pm", bufs=2,
                                   name=f"pmB{j}_{fc}")
                    for g in range(4):
                        pm = pmA if g < 2 else pmB
                        nc.tensor.matmul(
                            pm[:, g % 2, :],
                            lhsT=oh_sb[32 * g:32 * g + 32,
                                       j * 128:(j + 1) * 128],
                            rhs=par_sb[32 * g:32 * g + 32,
                                       fc * 512:(fc + 1) * 512],
                            start=True, stop=True,
                            tile_position=(32 * g, 0))
                    nc.scalar.activation(
                        E_sb[:, 4 * j:4 * j + 2, fc * 512:(fc + 1) * 512],
                        pmA[:], AFT.Exp)
                    nc.scalar.activation(
                        E_sb[:, 4 * j + 2:4 * j + 4, fc * 512:(fc + 1) * 512],
                        pmB[:], AFT.Exp)
                    if j > 0:
                        z_mms(j - 1, (fc,))
            z_mms(7, (0, 1))
            nc.vector.reciprocal(invz_bc[:], zz[:])

        if stage == "1":
            # bisect: dump E k-subtiles bt-shaped (out rows b <-> subtile b)
            for bt in range(NBT):
                prof = res.tile([128, FL], F32, tag="prof1", bufs=2,
                                name=f"p1_{bt}")
                nc.scalar.copy(prof[:], E_sb[:, 4 * bt, :])
                nc.sync.dma_start(out[bt * 128:(bt + 1) * 128, :], prof[:])
            return

        # ---------- phase B: U = freq @ E^T; fused 1/Z scale + rowsum ----
        pooled = res.tile([128, NBT * FL], F32)
        s_col = res.tile([128, NBT], F32)
        s_sum = res.tile([128, NBT], F32)
        rinv = res.tile([128, NBT], F32)

        s_in1 = dram.tile([128, SPLIT_BT], F32)
        s_out1 = dram.tile([NCORES * 128, SPLIT_BT], F32, addr_space="Shared")
        s_in2 = dram.tile([128, NBT - SPLIT_BT], F32)
        s_out2 = dram.tile([NCORES * 128, NBT - SPLIT_BT], F32,
                           addr_space="Shared")
        sgat1 = res.tile([128, NCORES * SPLIT_BT], F32)
        sgat2 = res.tile([128, NCORES * (NBT - SPLIT_BT)], F32)

        no_cc = bool(os.environ.get("KERNEL_NO_COLLECTIVE"))

        if not no_cc:
            # tiny warm-up AllGather: absorbs the ncfw cold-start latency so
            # the two real collectives start promptly (output is unused)
            wu_in = dram.tile([128, 1], F32)
            wu_out = dram.tile([NCORES * 128, 1], F32, addr_space="Shared")
            nc.sync.dma_start(wu_in[:], invz_bc[:, 0:1])
            nc.gpsimd.collective_compute(
                "AllGather", ALU.bypass,
                replica_groups=[list(range(NCORES))],
                ins=[wu_in.opt()], outs=[wu_out.opt()])

        def emit_collective(lo, hi, s_in, s_out, sgat):
            cnt = hi - lo
            if no_cc:
                nc.vector.tensor_scalar_mul(s_sum[:, lo:hi], s_col[:, lo:hi],
                                            float(NCORES))
            else:
                nc.sync.dma_start(s_in[:], s_col[:, lo:hi])
                nc.gpsimd.collective_compute(
                    "AllGather", ALU.bypass,
                    replica_groups=[list(range(NCORES))],
                    ins=[s_in.opt()], outs=[s_out.opt()])
                for r in range(NCORES):
                    nc.sync.dma_start(sgat[:, r * cnt:(r + 1) * cnt],
                                      s_out[r * 128:(r + 1) * 128, :])
                nc.vector.tensor_add(sgat[:, 0:4 * cnt], sgat[:, 0:4 * cnt],
                                     sgat[:, 4 * cnt:8 * cnt])
                nc.vector.tensor_add(sgat[:, 0:2 * cnt], sgat[:, 0:2 * cnt],
                                     sgat[:, 2 * cnt:4 * cnt])
                nc.vector.tensor_add(s_sum[:, lo:hi], sgat[:, 0:cnt],
                                     sgat[:, cnt:2 * cnt])
            nc.vector.reciprocal(rinv[:, lo:hi], s_sum[:, lo:hi])

        with tc.tile_pool(name="psB", bufs=2, space="PSUM") as psB:
            for bt in range(NBT):
                pu = psB.tile([128, FL], F32, tag="pu", name=f"pu{bt}")
                for kp in range(KP):
                    for fc in range(2):
                        if USE_FP8:
                            nc.tensor.matmul(
                                pu[:, fc * 512:(fc + 1) * 512],
                                lhsT=freq_sb[:, 2 * kp:2 * kp + 2,
                                             bt * 128:(bt + 1) * 128],
                                rhs=E_sb[:, 2 * kp:2 * kp + 2,
                                         fc * 512:(fc + 1) * 512],
                                start=(kp == 0), stop=(kp == KP - 1),
                                perf_mode=DR)
                        else:
                            for k in (2 * kp, 2 * kp + 1):
                                nc.tensor.matmul(
                                    pu[:, fc * 512:(fc + 1) * 512],
                                    lhsT=freq_sb[:, k, bt * 128:(bt + 1) * 128],
                                    rhs=E_sb[:, k, fc * 512:(fc + 1) * 512],
                                    start=(k == 0), stop=(k == MT - 1))
                if stage == "2":
                    nc.scalar.copy(pooled[:, bt * FL:(bt + 1) * FL], pu[:])
                    nc.sync.dma_start(out[bt * 128:(bt + 1) * 128, :],
                                      pooled[:, bt * FL:(bt + 1) * FL])
                    continue
                if os.environ.get("KERNEL_TTRED", "") not in ("", "0"):
                    nc.vector.tensor_tensor_reduce(
                        out=pooled[:, bt * FL:(bt + 1) * FL],
                        in0=pu[:], in1=invz_bc[:],
                        scale=1.0, scalar=0.0,
                        op0=ALU.mult, op1=ALU.add,
                        accum_out=s_col[:, bt:bt + 1])
                else:
                    nc.vector.tensor_mul(pooled[:, bt * FL:(bt + 1) * FL],
                                         pu[:], invz_bc[:])
                    nc.vector.reduce_sum(s_col[:, bt:bt + 1],
                                         pooled[:, bt * FL:(bt + 1) * FL],
                                         axis=mybir.AxisListType.X)
                if bt == SPLIT_BT - 1:
                    emit_collective(0, SPLIT_BT, s_in1, s_out1, sgat1)
            if stage == "2":
                return
            emit_collective(SPLIT_BT, NBT, s_in2, s_out2, sgat2)

            # ---------- profile = pooled * (1/s); write out ----------
            for bt in range(NBT):
                sl = pooled[:, bt * FL:(bt + 1) * FL]
                if stage == "3":
                    pass  # skip rinv scaling: dump pooled
                elif bt % 2 == 0:
                    nc.scalar.activation(sl, sl, AFT.Copy,
                                         scale=rinv[:, bt:bt + 1])
                else:
                    nc.vector.tensor_scalar_mul(sl, sl, rinv[:, bt:bt + 1])
                nc.sync.dma_start(out[bt * 128:(bt + 1) * 128, :], sl)


def _build_bass():
    nc = bacc.Bacc("TRN2", target_bir_lowering=False, debug=False,
                   num_devices=NCORES)
    idt = FP8 if USE_FP8 else BF16
    freqT = nc.dram_tensor("freqT", [M, B], idt, kind="ExternalInput").ap()
    oh4 = nc.dram_tensor("oh4", [128, 8 * 128], BF16, kind="ExternalInput").ap()
    par4 = nc.dram_tensor("par4", [128, FL], BF16, kind="ExternalInput").ap()
    out = nc.dram_tensor("out", [B, FL], F32, kind="ExternalOutput").ap()

    with tile.TileContext(nc) as tc:
        _body(tc, freqT, oh4, par4, out)
    nc.compile()
    return nc


def _get_nc():
    if "nc" not in _CACHE:
        _CACHE["nc"] = _build_bass()
    return _CACHE["nc"]


def _prepare_in_maps(freq, kmer_params, temperature, kmer_idcs):
    freq = np.asarray(freq, dtype=np.float32)            # (B, M)
    kp = np.asarray(kmer_params, dtype=np.float64)       # (F, 4, K)
    temp = float(np.asarray(temperature, dtype=np.float64).reshape(-1)[0])
    idcs = np.asarray(kmer_idcs).astype(np.int64)        # (M, K)

    assert freq.shape == (B, M) and kp.shape == (F, NBASE, KMER)
    assert idcs.shape == (M, KMER)

    # params_eff folds 1/T, the per-(f, j) max shift (softmax-invariant) and
    # ln(128)/K so that E' = exp(matches_eff) lies in (0, 128].
    shift = kp.max(axis=1) / temp                        # (F, K)
    scale_ln = np.log(128.0) / KMER if USE_FP8 else 0.0
    pf = (kp / temp - shift[:, None, :] + scale_ln)      # (F, 4, K)
    pf_flat = pf.reshape(F, KK).astype(np.float32)       # [f, c*K + j]

    # onehot^T of the kmer index input: ohT[c*K+j, i] = 1 iff idcs[i, j] == c
    onehot = np.zeros((M, NBASE, KMER), dtype=np.float32)
    onehot[np.arange(M)[:, None], idcs, np.arange(KMER)[None, :]] = 1.0
    ohT = onehot.reshape(M, KK).T                        # (24, M)

    # 4-row packing: row group g handles m-tiles t = 4j + g
    oh4 = np.zeros((128, 8, 128), dtype=np.float32)
    for g in range(NBASE):
        for j in range(8):
            t = 4 * j + g
            oh4[32 * g:32 * g + KK, j, :] = ohT[:, t * 128:(t + 1) * 128]
    oh4 = np.ascontiguousarray(
        oh4.reshape(128, 8 * 128)).astype(ml_dtypes.bfloat16)

    if USE_FP8:
        freqT = np.ascontiguousarray(freq.T * 128.0).astype(
            ml_dtypes.float8_e4m3)
    else:
        freqT = np.ascontiguousarray(freq.T).astype(ml_dtypes.bfloat16)

    in_maps = []
    for c in range(NCORES):
        pfc = pf_flat[c * FL:(c + 1) * FL]               # (FL, 24)
        par4 = np.zeros((128, FL), dtype=np.float32)
        for g in range(NBASE):
            par4[32 * g:32 * g + KK, :] = pfc.T
        in_maps.append({
            "freqT": freqT,
            "oh4": oh4,
            "par4": np.ascontiguousarray(par4).astype(ml_dtypes.bfloat16),
        })
    return in_maps


def _run(in_maps, trace=False):
    nc = _get_nc()
    return run_bass_kernel_spmd(nc, in_maps, list(range(NCORES)), trace=trace)


def kernel(freq, kmer_params, temperature, kmer_idcs):
    in_maps = _prepare_in_maps(freq, kmer_params, temperature, kmer_idcs)
    res = _run(in_maps,
               trace=os.environ.get("KERNEL_TRACE", "") not in ("", "0"))
    _CACHE["last_result"] = res
    return np.concatenate(
        [np.asarray(res.results[c]["out"], dtype=np.float32)
         for c in range(NCORES)], axis=1)


# revision 10
# speedup vs baseline: 1.1847x; 1.1847x over previous
"""Trainium2 Bass kernel for the ConvFeatureExtractor problem.

Reference computation (all f32):
    matches[f, i] = sum_j kmer_params[f, kmer_idcs[i, j], j]      # (F, M)
    probs = softmax(matches / temperature, axis=1)                # over M
    pooled = freq @ probs.T                                       # (B, F)
    profile = pooled / pooled.sum(axis=1, keepdims=True)

Shapes: B=1024, M=4096 (=4^6 kmers), F=8192 filters, K=6, 4 bases.

Kernel strategy (8 NeuronCores, filter-sharded: FL = F/8 = 1024 per core):
  * Host folds 1/T, the per-(filter,position) max-shift and a x128 scale
    into params_eff; softmax is invariant to the shift, and the scale
    cancels in the final normalization.  E' = exp(matches_eff) lands in
    (0, 128] which fits fp8e4 (max 240) with all mass in normal range.
  * matches^T via K=24 matmuls, 4-row-packed into the PE array
    (tile_position row groups), exp on ScalarE written as fp8 E.
  * Z[f] = sum_i E[i, f] via DoubleRow ones-matmuls interleaved with
    phase A (broadcast across partitions for free).
  * U = freq @ E^T as fp8 DoubleRow matmuls (2 MACs/cell/cycle).
  * One fused DVE tensor_tensor_reduce per batch tile applies 1/Z and
    produces the per-row sums s in the same pass.
  * s AllReduce over the 8 cores split in two chunks so the first chunk's
    collective latency hides under the tail of the main GEMM.
Each core returns its (B, FL) f32 slice; host concatenates along F.
"""

import os

import numpy as np
import ml_dtypes

import concourse.bass as bass  # noqa: F401
import concourse.tile as tile
from concourse import bacc, mybir
from concourse.bass_utils import run_bass_kernel_spmd

NCORES = 8
B = 1024           # batch
M = 4096           # 4^6 kmers
F = 8192           # filters
KMER = 6           # kmer length
NBASE = 4
KK = NBASE * KMER  # 24 flattened (base, position)
FL = F // NCORES   # 1024 filters per core

MT = M // 128      # 32 contraction subtiles of 128
KP = MT // 2       # 16 DoubleRow k-pairs
NBT = B // 128     # 8 batch tiles
SPLIT_BT = 4       # batch tiles covered by the first (overlapped) AllReduce

BF16 = mybir.dt.bfloat16
FP8 = mybir.dt.float8e4
F32 = mybir.dt.float32
AFT = mybir.ActivationFunctionType
ALU = mybir.AluOpType
DR = mybir.MatmulPerfMode.DoubleRow

USE_FP8 = os.environ.get("KERNEL_BF16", "") in ("", "0")

_CACHE: dict = {}


def _body(tc, freqT, oh4, par4, out):
    nc = tc.nc
    stage = os.environ.get("KERNEL_STAGE", "")
    edt = FP8 if USE_FP8 else BF16
    with (
        tc.tile_pool(name="res", bufs=1) as res,
        tc.tile_pool(name="dram", bufs=1, space="DRAM") as dram,
    ):
        # ---------- constants / small inputs ----------
        oh_sb = res.tile([128, 8 * 128], BF16)      # 4-row-packed onehot^T
        nc.sync.dma_start(oh_sb[:], oh4[:])
        par_sb = res.tile([128, FL], BF16)          # 4-row-packed params_eff^T
        nc.sync.dma_start(par_sb[:], par4[:])
        ones_e = res.tile([128, 2, 128], edt)       # DoubleRow ones lhsT
        nc.vector.memset(ones_e[:], 1.0)
        # memset-initialized rhs for PE warm-up matmuls (no DMA dependency)
        warm_rhs = res.tile([128, 2, 512], edt)
        nc.vector.memset(warm_rhs[:], 1.0)

        # ---------- stream in freq^T (fp8/bf16, k-subtile major) ----------
        freq_sb = res.tile([128, MT, B], edt)
        for k in range(MT):
            nc.sync.dma_start(freq_sb[:, k, :], freqT[k * 128:(k + 1) * 128, :])

        E_sb = res.tile([128, MT, FL], edt)
        invz_bc = res.tile([128, FL], F32)

        # ---------- phase A: matches -> E = exp(.); Z via ones-matmuls ----
        with tc.tile_pool(name="psA", bufs=1, space="PSUM") as psA:
            zz = psA.tile([128, FL], F32, tag="zz")

            # ---- PE warm-up: dep-free matmuls fill the input-DMA head so
            # the HAM clock gate reaches 8/8 before real compute starts.
            warm_ps = psA.tile([128, 512], F32, tag="warm")
            for i in range(24):
                if USE_FP8:
                    nc.tensor.matmul(warm_ps[:], lhsT=ones_e[:],
                                     rhs=warm_rhs[:], start=True, stop=True,
                                     perf_mode=DR)
                else:
                    nc.tensor.matmul(warm_ps[:], lhsT=ones_e[:, 0, :],
                                     rhs=warm_rhs[:, 0, :],
                                     start=True, stop=True)

            def z_mms(j, fcs):
                # Z accumulation for the pack produced in iteration j
                for kp, fc in ((kpj, fcj) for kpj in (2 * j, 2 * j + 1)
                               for fcj in fcs):
                    if USE_FP8:
                        nc.tensor.matmul(
                            zz[:, fc * 512:(fc + 1) * 512],
                            lhsT=ones_e[:],
                            rhs=E_sb[:, 2 * kp:2 * kp + 2,
                                     fc * 512:(fc + 1) * 512],
                            start=(kp == 0), stop=(kp == KP - 1),
                            perf_mode=DR)
                    else:
                        for k in (2 * kp, 2 * kp + 1):
                            nc.tensor.matmul(
                                zz[:, fc * 512:(fc + 1) * 512],
                                lhsT=ones_e[:, 0, :],
                                rhs=E_sb[:, k, fc * 512:(fc + 1) * 512],
                                start=(k == 0), stop=(k == MT - 1))

            for j in range(8):          # packs of 4 m-tiles (t = 4j+g)
                for fc in range(2):
                    pmA = psA.tile([128, 2, 512], F32, tag="pm", bufs=2,
                                   name=f"pmA{j}_{fc}")
                    pmB = psA.tile([128, 2, 512], F32, tag="pm", bufs=2,
                                   name=f"pmB{j}_{fc}")
                    for g in range(4):
                        pm = pmA if g < 2 else pmB
                        nc.tensor.matmul(
                            pm[:, g % 2, :],
                            lhsT=oh_sb[32 * g:32 * g + 32,
                                       j * 128:(j + 1) * 128],
                            rhs=par_sb[32 * g:32 * g + 32,
                                       fc * 512:(fc + 1) * 512],
                            start=True, stop=True,
                            tile_position=(32 * g, 0))
                    nc.scalar.activation(
                        E_sb[:, 4 * j:4 * j + 2, fc * 512:(fc + 1) * 512],
                        pmA[:], AFT.Exp)
                    nc.scalar.activation(
                        E_sb[:, 4 * j + 2:4 * j + 4, fc * 512:(fc + 1) * 512],
                        pmB[:], AFT.Exp)
                    if j > 0:
                        z_mms(j - 1, (fc,))
                    for _ in range(3):   # HAM keep-warm filler
                        if USE_FP8:
                            nc.tensor.matmul(warm_ps[:], lhsT=ones_e[:],
                                             rhs=warm_rhs[:], start=True,
                                             stop=True, perf_mode=DR)
                        else:
                            nc.tensor.matmul(warm_ps[:], lhsT=ones_e[:, 0, :],
                                             rhs=warm_rhs[:, 0, :],
                                             start=True, stop=True)
            z_mms(7, (0, 1))
            nc.vector.reciprocal(invz_bc[:], zz[:])

        if stage == "1":
            # bisect: dump E k-subtiles bt-shaped (out rows b <-> subtile b)
            for bt in range(NBT):
                prof = res.tile([128, FL], F32, tag="prof1", bufs=2,
                                name=f"p1_{bt}")
                nc.scalar.copy(prof[:], E_sb[:, 4 * bt, :])
                nc.sync.dma_start(out[bt * 128:(bt + 1) * 128, :], prof[:])
            return

        # ---------- phase B: U = freq @ E^T; fused 1/Z scale + rowsum ----
        pooled = res.tile([128, NBT * FL], F32)
        s_col = res.tile([128, NBT], F32)
        s_sum = res.tile([128, NBT], F32)
        rinv = res.tile([128, NBT], F32)

        s_in1 = dram.tile([128, SPLIT_BT], F32)
        s_out1 = dram.tile([128, SPLIT_BT], F32, addr_space="Shared")
        s_in2 = dram.tile([128, NBT - SPLIT_BT], F32)
        s_out2 = dram.tile([128, NBT - SPLIT_BT], F32,
                           addr_space="Shared")

        no_cc = bool(os.environ.get("KERNEL_NO_COLLECTIVE"))

        def emit_collective(lo, hi, s_in, s_out, sgat):
            if no_cc:
                nc.vector.tensor_scalar_mul(s_sum[:, lo:hi], s_col[:, lo:hi],
                                            float(NCORES))
            else:
                nc.sync.dma_start(s_in[:], s_col[:, lo:hi])
                nc.gpsimd.collective_compute(
                    "AllReduce", ALU.add,
                    replica_groups=[list(range(NCORES))],
                    ins=[s_in.opt()], outs=[s_out.opt()])
                nc.sync.dma_start(s_sum[:, lo:hi], s_out[:])
            nc.vector.reciprocal(rinv[:, lo:hi], s_sum[:, lo:hi])

        with tc.tile_pool(name="psB", bufs=2, space="PSUM") as psB:
            for bt in range(NBT):
                pu = psB.tile([128, FL], F32, tag="pu", name=f"pu{bt}")
                for kp in range(KP):
                    for fc in range(2):
                        if USE_FP8:
                            nc.tensor.matmul(
                                pu[:, fc * 512:(fc + 1) * 512],
                                lhsT=freq_sb[:, 2 * kp:2 * kp + 2,
                                             bt * 128:(bt + 1) * 128],
                                rhs=E_sb[:, 2 * kp:2 * kp + 2,
                                         fc * 512:(fc + 1) * 512],
                                start=(kp == 0), stop=(kp == KP - 1),
                                perf_mode=DR)
                        else:
                            for k in (2 * kp, 2 * kp + 1):
                                nc.tensor.matmul(
                                    pu[:, fc * 512:(fc + 1) * 512],
                                    lhsT=freq_sb[:, k, bt * 128:(bt + 1) * 128],
                                    rhs=E_sb[:, k, fc * 512:(fc + 1) * 512],
                                    start=(k == 0), stop=(k == MT - 1))
                if stage == "2":
                    nc.scalar.copy(pooled[:, bt * FL:(bt + 1) * FL], pu[:])
                    nc.sync.dma_start(out[bt * 128:(bt + 1) * 128, :],
                                      pooled[:, bt * FL:(bt + 1) * FL])
                    continue
                if os.environ.get("KERNEL_TTRED", "") not in ("", "0"):
                    nc.vector.tensor_tensor_reduce(
                        out=pooled[:, bt * FL:(bt + 1) * FL],
                        in0=pu[:], in1=invz_bc[:],
                        scale=1.0, scalar=0.0,
                        op0=ALU.mult, op1=ALU.add,
                        accum_out=s_col[:, bt:bt + 1])
                else:
                    nc.vector.tensor_mul(pooled[:, bt * FL:(bt + 1) * FL],
                                         pu[:], invz_bc[:])
                    nc.vector.reduce_sum(s_col[:, bt:bt + 1],
                                         pooled[:, bt * FL:(bt + 1) * FL],
                                         axis=mybir.AxisListType.X)
                if bt == SPLIT_BT - 1:
                    emit_collective(0, SPLIT_BT, s_in1, s_out1, None)
            if stage == "2":
                return
            emit_collective(SPLIT_BT, NBT, s_in2, s_out2, None)

            # ---------- profile = pooled * (1/s); write out ----------
            for bt in range(NBT):
                sl = pooled[:, bt * FL:(bt + 1) * FL]
                if stage == "3":
                    pass  # skip rinv scaling: dump pooled
                elif bt % 2 == 0:
                    nc.scalar.activation(sl, sl, AFT.Copy,
                                         scale=rinv[:, bt:bt + 1])
                else:
                    nc.vector.tensor_scalar_mul(sl, sl, rinv[:, bt:bt + 1])
                nc.sync.dma_start(out[bt * 128:(bt + 1) * 128, :], sl)


def _build_bass():
    nc = bacc.Bacc("TRN2", target_bir_lowering=False, debug=False,
                   num_devices=NCORES)
    idt = FP8 if USE_FP8 else BF16
    freqT = nc.dram_tensor("freqT", [M, B], idt, kind="ExternalInput").ap()
    oh4 = nc.dram_tensor("oh4", [128, 8 * 128], BF16, kind="ExternalInput").ap()
    par4 = nc.dram_tensor("par4", [128, FL], BF16, kind="ExternalInput").ap()
    out = nc.dram_tensor("out", [B, FL], F32, kind="ExternalOutput").ap()

    with tile.TileContext(nc) as tc:
        _body(tc, freqT, oh4, par4, out)
    nc.compile()
    return nc


def _get_nc():
    if "nc" not in _CACHE:
        _CACHE["nc"] = _build_bass()
    return _CACHE["nc"]


def _prepare_in_maps(freq, kmer_params, temperature, kmer_idcs):
    freq = np.asarray(freq, dtype=np.float32)            # (B, M)
    kp = np.asarray(kmer_params, dtype=np.float64)       # (F, 4, K)
    temp = float(np.asarray(temperature, dtype=np.float64).reshape(-1)[0])
    idcs = np.asarray(kmer_idcs).astype(np.int64)        # (M, K)

    assert freq.shape == (B, M) and kp.shape == (F, NBASE, KMER)
    assert idcs.shape == (M, KMER)

    # params_eff folds 1/T, the per-(f, j) max shift (softmax-invariant) and
    # ln(128)/K so that E' = exp(matches_eff) lies in (0, 128].
    shift = kp.max(axis=1) / temp                        # (F, K)
    scale_ln = np.log(128.0) / KMER if USE_FP8 else 0.0
    pf = (kp / temp - shift[:, None, :] + scale_ln)      # (F, 4, K)
    pf_flat = pf.reshape(F, KK).astype(np.float32)       # [f, c*K + j]

    # onehot^T of the kmer index input: ohT[c*K+j, i] = 1 iff idcs[i, j] == c
    onehot = np.zeros((M, NBASE, KMER), dtype=np.float32)
    onehot[np.arange(M)[:, None], idcs, np.arange(KMER)[None, :]] = 1.0
    ohT = onehot.reshape(M, KK).T                        # (24, M)

    # 4-row packing: row group g handles m-tiles t = 4j + g
    oh4 = np.zeros((128, 8, 128), dtype=np.float32)
    for g in range(NBASE):
        for j in range(8):
            t = 4 * j + g
            oh4[32 * g:32 * g + KK, j, :] = ohT[:, t * 128:(t + 1) * 128]
    oh4 = np.ascontiguousarray(
        oh4.reshape(128, 8 * 128)).astype(ml_dtypes.bfloat16)

    if USE_FP8:
        freqT = np.ascontiguousarray(freq.T * 128.0).astype(
            ml_dtypes.float8_e4m3)
    else:
        freqT = np.ascontiguousarray(freq.T).astype(ml_dtypes.bfloat16)

    in_maps = []
    for c in range(NCORES):
        pfc = pf_flat[c * FL:(c + 1) * FL]               # (FL, 24)
        par4 = np.zeros((128, FL), dtype=np.float32)
        for g in range(NBASE):
            par4[32 * g:32 * g + KK, :] = pfc.T
        in_maps.append({
            "freqT": freqT,
            "oh4": oh4,
            "par4": np.ascontiguousarray(par4).astype(ml_dtypes.bfloat16),
        })
    return in_maps


def _run(in_maps, trace=False):
    nc = _get_nc()
    return run_bass_kernel_spmd(nc, in_maps, list(range(NCORES)), trace=trace)


def kernel(freq, kmer_params, temperature, kmer_idcs):
    in_maps = _prepare_in_maps(freq, kmer_params, temperature, kmer_idcs)
    res = _run(in_maps,
               trace=os.environ.get("KERNEL_TRACE", "") not in ("", "0"))
    _CACHE["last_result"] = res
    return np.concatenate(
        [np.asarray(res.results[c]["out"], dtype=np.float32)
         for c in range(NCORES)], axis=1)


# revision 13
# speedup vs baseline: 1.2656x; 1.0682x over previous
"""Trainium2 Bass kernel for the ConvFeatureExtractor problem.

Reference computation (all f32):
    matches[f, i] = sum_j kmer_params[f, kmer_idcs[i, j], j]      # (F, M)
    probs = softmax(matches / temperature, axis=1)                # over M
    pooled = freq @ probs.T                                       # (B, F)
    profile = pooled / pooled.sum(axis=1, keepdims=True)

Shapes: B=1024, M=4096 (=4^6 kmers), F=8192 filters, K=6, 4 bases.

Kernel strategy (8 NeuronCores, filter-sharded: FL = F/8 = 1024 per core):
  * Host folds 1/T, the per-(filter,position) max-shift and a x128 scale
    into params_eff; softmax is invariant to the shift, and the scale
    cancels in the final normalization.  E' = exp(matches_eff) lands in
    (0, 128] which fits fp8e4 (max 240) with all mass in normal range.
  * matches^T via K=24 matmuls, 4-row-packed into the PE array
    (tile_position row groups), exp on ScalarE written as fp8 E.
  * Z[f] = sum_i E[i, f] via DoubleRow ones-matmuls interleaved with
    phase A (broadcast across partitions for free).
  * U = freq @ E^T as fp8 DoubleRow matmuls (2 MACs/cell/cycle).
  * One fused DVE tensor_tensor_reduce per batch tile applies 1/Z and
    produces the per-row sums s in the same pass.
  * s AllReduce over the 8 cores split in two chunks so the first chunk's
    collective latency hides under the tail of the main GEMM.
Each core returns its (B, FL) f32 slice; host concatenates along F.
"""

import os

import numpy as np
import ml_dtypes

import concourse.bass as bass  # noqa: F401
import concourse.tile as tile
from concourse import bacc, mybir
from concourse.bass_utils import run_bass_kernel_spmd

NCORES = 8
B = 1024           # batch
M = 4096           # 4^6 kmers
F = 8192           # filters
KMER = 6           # kmer length
NBASE = 4
KK = NBASE * KMER  # 24 flattened (base, position)
FL = F // NCORES   # 1024 filters per core

MT = M // 128      # 32 contraction subtiles of 128
KP = MT // 2       # 16 DoubleRow k-pairs
NBT = B // 128     # 8 batch tiles
SPLIT_BT = 4       # batch tiles covered by the first (overlapped) AllReduce

BF16 = mybir.dt.bfloat16
FP8 = mybir.dt.float8e4
F32 = mybir.dt.float32
AFT = mybir.ActivationFunctionType
ALU = mybir.AluOpType
DR = mybir.MatmulPerfMode.DoubleRow

USE_FP8 = os.environ.get("KERNEL_BF16", "") in ("", "0")

_CACHE: dict = {}


def _body(tc, freqT, oh4, par4, out):
    nc = tc.nc
    stage = os.environ.get("KERNEL_STAGE", "")
    edt = FP8 if USE_FP8 else BF16
    with (
        tc.tile_pool(name="res", bufs=1) as res,
        tc.tile_pool(name="dram", bufs=1, space="DRAM") as dram,
    ):
        # ---------- constants / small inputs ----------
        oh_sb = res.tile([128, 8 * 128], BF16)      # 4-row-packed onehot^T
        nc.sync.dma_start(oh_sb[:], oh4[:])
        par_sb = res.tile([128, FL], BF16)          # 4-row-packed params_eff^T
        nc.sync.dma_start(par_sb[:], par4[:])
        ones_e = res.tile([128, 2, 128], edt)       # DoubleRow ones lhsT
        nc.vector.memset(ones_e[:], 1.0)
        # memset-initialized rhs for PE warm-up matmuls (no DMA dependency)
        warm_rhs = res.tile([128, 2, 512], edt)
        nc.vector.memset(warm_rhs[:], 1.0)

        # ---------- stream in freq^T (fp8/bf16, k-subtile major) ----------
        freq_sb = res.tile([128, MT, B], edt)
        for k in range(MT):
            nc.sync.dma_start(freq_sb[:, k, :], freqT[k * 128:(k + 1) * 128, :])

        E_sb = res.tile([128, MT, FL], edt)
        invz_bc = res.tile([128, FL], F32)
        pooled = res.tile([128, NBT * FL], F32)
        s_col = res.tile([128, NBT], F32)

        # single PSUM pool: pm 2x[128,2,512] + pu 2x[128,1024] = 8 banks
        with tc.tile_pool(name="ps", bufs=1, space="PSUM") as ps:
            zz = ps.tile([128, FL], F32, tag="pu", bufs=2)
            pu0 = ps.tile([128, FL], F32, tag="pu", bufs=2)

            # ---- PE warm-up: dep-free matmuls fill the input-DMA head so
            # the HAM clock gate reaches 8/8 before real compute starts.
            # They target zz with start/stop groups; the real Z accumulation
            # restarts the bank with start=True so the values are discarded.
            for i in range(24):
                if USE_FP8:
                    nc.tensor.matmul(zz[:, 0:512], lhsT=ones_e[:],
                                     rhs=warm_rhs[:], start=True, stop=True,
                                     perf_mode=DR)
                else:
                    nc.tensor.matmul(zz[:, 0:512], lhsT=ones_e[:, 0, :],
                                     rhs=warm_rhs[:, 0, :],
                                     start=True, stop=True)

            def bt_mms(bt, pu, kps, fcs):
                # main-GEMM contributions for batch tile bt, k-pairs kps
                for kp in kps:
                    for fc in fcs:
                        if USE_FP8:
                            nc.tensor.matmul(
                                pu[:, fc * 512:(fc + 1) * 512],
                                lhsT=freq_sb[:, 2 * kp:2 * kp + 2,
                                             bt * 128:(bt + 1) * 128],
                                rhs=E_sb[:, 2 * kp:2 * kp + 2,
                                         fc * 512:(fc + 1) * 512],
                                start=(kp == 0), stop=(kp == KP - 1),
                                perf_mode=DR)
                        else:
                            for k in (2 * kp, 2 * kp + 1):
                                nc.tensor.matmul(
                                    pu[:, fc * 512:(fc + 1) * 512],
                                    lhsT=freq_sb[:, k, bt * 128:(bt + 1) * 128],
                                    rhs=E_sb[:, k, fc * 512:(fc + 1) * 512],
                                    start=(k == 0), stop=(k == MT - 1))

            def z_mms(j, fcs):
                # Z accumulation for the pack produced in iteration j
                for kp, fc in ((kpj, fcj) for kpj in (2 * j, 2 * j + 1)
                               for fcj in fcs):
                    if USE_FP8:
                        nc.tensor.matmul(
                            zz[:, fc * 512:(fc + 1) * 512],
                            lhsT=ones_e[:],
                            rhs=E_sb[:, 2 * kp:2 * kp + 2,
                                     fc * 512:(fc + 1) * 512],
                            start=(kp == 0), stop=(kp == KP - 1),
                            perf_mode=DR)
                    else:
                        for k in (2 * kp, 2 * kp + 1):
                            nc.tensor.matmul(
                                zz[:, fc * 512:(fc + 1) * 512],
                                lhsT=ones_e[:, 0, :],
                                rhs=E_sb[:, k, fc * 512:(fc + 1) * 512],
                                start=(k == 0), stop=(k == MT - 1))

            for j in range(8):          # packs of 4 m-tiles (t = 4j+g)
                for fc in range(2):
                    pmA = ps.tile([128, 2, 512], F32, tag="pm", bufs=2,
                                   name=f"pmA{j}_{fc}")
                    pmB = ps.tile([128, 2, 512], F32, tag="pm", bufs=2,
                                   name=f"pmB{j}_{fc}")
                    for g in range(4):
                        pm = pmA if g < 2 else pmB
                        nc.tensor.matmul(
                            pm[:, g % 2, :],
                            lhsT=oh_sb[32 * g:32 * g + 32,
                                       j * 128:(j + 1) * 128],
                            rhs=par_sb[32 * g:32 * g + 32,
                                       fc * 512:(fc + 1) * 512],
                            start=True, stop=True,
                            tile_position=(32 * g, 0))
                    nc.scalar.activation(
                        E_sb[:, 4 * j:4 * j + 2, fc * 512:(fc + 1) * 512],
                        pmA[:], AFT.Exp)
                    nc.scalar.activation(
                        E_sb[:, 4 * j + 2:4 * j + 4, fc * 512:(fc + 1) * 512],
                        pmB[:], AFT.Exp)
                    if j > 0:
                        z_mms(j - 1, (fc,))
                        bt_mms(0, pu0, (2 * (j - 1), 2 * j - 1), (fc,))
            z_mms(7, (0, 1))
            bt_mms(0, pu0, (14, 15), (0, 1))
            nc.vector.reciprocal(invz_bc[:], zz[:])

            if stage == "1":
                # bisect: dump E k-subtiles bt-shaped
                for bt in range(NBT):
                    prof = res.tile([128, FL], F32, tag="prof1", bufs=2,
                                    name=f"p1_{bt}")
                    nc.scalar.copy(prof[:], E_sb[:, 4 * bt, :])
                    nc.sync.dma_start(out[bt * 128:(bt + 1) * 128, :], prof[:])
                return

            # ------ phase B: U = freq @ E^T; 1/Z scale + rowsum ------
            s_sum = res.tile([128, NBT], F32)
            rinv = res.tile([128, NBT], F32)

            s_in1 = dram.tile([128, SPLIT_BT], F32)
            s_out1 = dram.tile([128, SPLIT_BT], F32, addr_space="Shared")
            s_in2 = dram.tile([128, NBT - SPLIT_BT], F32)
            s_out2 = dram.tile([128, NBT - SPLIT_BT], F32,
                               addr_space="Shared")

            no_cc = bool(os.environ.get("KERNEL_NO_COLLECTIVE"))

            def emit_collective(lo, hi, s_in, s_out):
                if no_cc:
                    nc.vector.tensor_scalar_mul(s_sum[:, lo:hi],
                                                s_col[:, lo:hi],
                                                float(NCORES))
                else:
                    nc.sync.dma_start(s_in[:], s_col[:, lo:hi])
                    nc.gpsimd.collective_compute(
                        "AllReduce", ALU.add,
                        replica_groups=[list(range(NCORES))],
                        ins=[s_in.opt()], outs=[s_out.opt()])
                    nc.sync.dma_start(s_sum[:, lo:hi], s_out[:])
                nc.vector.reciprocal(rinv[:, lo:hi], s_sum[:, lo:hi])

            def bt_epilogue(bt, pu):
                if stage == "2":
                    nc.scalar.copy(pooled[:, bt * FL:(bt + 1) * FL], pu[:])
                    nc.sync.dma_start(out[bt * 128:(bt + 1) * 128, :],
                                      pooled[:, bt * FL:(bt + 1) * FL])
                    return
                nc.vector.tensor_mul(pooled[:, bt * FL:(bt + 1) * FL],
                                     pu[:], invz_bc[:])
                nc.vector.reduce_sum(s_col[:, bt:bt + 1],
                                     pooled[:, bt * FL:(bt + 1) * FL],
                                     axis=mybir.AxisListType.X)
                if bt == SPLIT_BT - 1:
                    emit_collective(0, SPLIT_BT, s_in1, s_out1)

            bt_epilogue(0, pu0)
            for bt in range(1, NBT):
                pu = ps.tile([128, FL], F32, tag="pu", bufs=2,
                             name=f"pu{bt}")
                bt_mms(bt, pu, range(KP), (0, 1))
                bt_epilogue(bt, pu)
            if stage == "2":
                return
            emit_collective(SPLIT_BT, NBT, s_in2, s_out2)

            # ---------- profile = pooled * (1/s); write out ----------
            for bt in range(NBT):
                sl = pooled[:, bt * FL:(bt + 1) * FL]
                if stage == "3":
                    pass  # skip rinv scaling: dump pooled
                elif bt % 2 == 0:
                    nc.scalar.activation(sl, sl, AFT.Copy,
                                         scale=rinv[:, bt:bt + 1])
                else:
                    nc.vector.tensor_scalar_mul(sl, sl, rinv[:, bt:bt + 1])
                nc.sync.dma_start(out[bt * 128:(bt + 1) * 128, :], sl)


def _build_bass():
    nc = bacc.Bacc("TRN2", target_bir_lowering=False, debug=False,
                   num_devices=NCORES)
    idt = FP8 if USE_FP8 else BF16
    freqT = nc.dram_tensor("freqT", [M, B], idt, kind="ExternalInput").ap()
    oh4 = nc.dram_tensor("oh4", [128, 8 * 128], BF16, kind="ExternalInput").ap()
    par4 = nc.dram_tensor("par4", [128, FL], BF16, kind="ExternalInput").ap()
    out = nc.dram_tensor("out", [B, FL], F32, kind="ExternalOutput").ap()

    with tile.TileContext(nc) as tc:
        _body(tc, freqT, oh4, par4, out)
    nc.compile()
    return nc


def _get_nc():
    if "nc" not in _CACHE:
        _CACHE["nc"] = _build_bass()
    return _CACHE["nc"]


def _prepare_in_maps(freq, kmer_params, temperature, kmer_idcs):
    freq = np.asarray(freq, dtype=np.float32)            # (B, M)
    kp = np.asarray(kmer_params, dtype=np.float64)       # (F, 4, K)
    temp = float(np.asarray(temperature, dtype=np.float64).reshape(-1)[0])
    idcs = np.asarray(kmer_idcs).astype(np.int64)        # (M, K)

    assert freq.shape == (B, M) and kp.shape == (F, NBASE, KMER)
    assert idcs.shape == (M, KMER)

    # params_eff folds 1/T, the per-(f, j) max shift (softmax-invariant) and
    # ln(128)/K so that E' = exp(matches_eff) lies in (0, 128].
    shift = kp.max(axis=1) / temp                        # (F, K)
    scale_ln = np.log(128.0) / KMER if USE_FP8 else 0.0
    pf = (kp / temp - shift[:, None, :] + scale_ln)      # (F, 4, K)
    pf_flat = pf.reshape(F, KK).astype(np.float32)       # [f, c*K + j]

    # onehot^T of the kmer index input: ohT[c*K+j, i] = 1 iff idcs[i, j] == c
    onehot = np.zeros((M, NBASE, KMER), dtype=np.float32)
    onehot[np.arange(M)[:, None], idcs, np.arange(KMER)[None, :]] = 1.0
    ohT = onehot.reshape(M, KK).T                        # (24, M)

    # 4-row packing: row group g handles m-tiles t = 4j + g
    oh4 = np.zeros((128, 8, 128), dtype=np.float32)
    for g in range(NBASE):
        for j in range(8):
            t = 4 * j + g
            oh4[32 * g:32 * g + KK, j, :] = ohT[:, t * 128:(t + 1) * 128]
    oh4 = np.ascontiguousarray(
        oh4.reshape(128, 8 * 128)).astype(ml_dtypes.bfloat16)

    if USE_FP8:
        freqT = np.ascontiguousarray(freq.T * 128.0).astype(
            ml_dtypes.float8_e4m3)
    else:
        freqT = np.ascontiguousarray(freq.T).astype(ml_dtypes.bfloat16)

    in_maps = []
    for c in range(NCORES):
        pfc = pf_flat[c * FL:(c + 1) * FL]               # (FL, 24)
        par4 = np.zeros((128, FL), dtype=np.float32)
        for g in range(NBASE):
            par4[32 * g:32 * g + KK, :] = pfc.T
        in_maps.append({
            "freqT": freqT,
            "oh4": oh4,
            "par4": np.ascontiguousarray(par4).astype(ml_dtypes.bfloat16),
        })
    return in_maps


def _run(in_maps, trace=False):
    nc = _get_nc()
    return run_bass_kernel_spmd(nc, in_maps, list(range(NCORES)), trace=trace)


def kernel(freq, kmer_params, temperature, kmer_idcs):
    in_maps = _prepare_in_maps(freq, kmer_params, temperature, kmer_idcs)
    res = _run(in_maps,
               trace=os.environ.get("KERNEL_TRACE", "") not in ("", "0"))
    _CACHE["last_result"] = res
    return np.concatenate(
        [np.asarray(res.results[c]["out"], dtype=np.float32)
         for c in range(NCORES)], axis=1)


# revision 14
# speedup vs baseline: 1.3914x; 1.0994x over previous
"""Trainium2 Bass kernel for the ConvFeatureExtractor problem.

Reference computation (all f32):
    matches[f, i] = sum_j kmer_params[f, kmer_idcs[i, j], j]      # (F, M)
    probs = softmax(matches / temperature, axis=1)                # over M
    pooled = freq @ probs.T                                       # (B, F)
    profile = pooled / pooled.sum(axis=1, keepdims=True)

Shapes: B=1024, M=4096 (=4^6 kmers), F=8192 filters, K=6, 4 bases.

Kernel strategy (8 NeuronCores, filter-sharded: FL = F/8 = 1024 per core):
  * Host folds 1/T, the per-(filter,position) max-shift and a x128 scale
    into params_eff; softmax is invariant to the shift, and the scale
    cancels in the final normalization.  E' = exp(matches_eff) lands in
    (0, 128] which fits fp8e4 (max 240) with all mass in normal range.
  * matches^T via K=24 matmuls, 4-row-packed into the PE array
    (tile_position row groups), exp on ScalarE written as fp8 E.
  * Z[f] = sum_i E[i, f] via DoubleRow ones-matmuls interleaved with
    phase A (broadcast across partitions for free).
  * U = freq @ E^T as fp8 DoubleRow matmuls (2 MACs/cell/cycle).
  * One fused DVE tensor_tensor_reduce per batch tile applies 1/Z and
    produces the per-row sums s in the same pass.
  * s AllReduce over the 8 cores split in two chunks so the first chunk's
    collective latency hides under the tail of the main GEMM.
Each core returns its (B, FL) f32 slice; host concatenates along F.
"""

import os

import numpy as np
import ml_dtypes

import concourse.bass as bass  # noqa: F401
import concourse.tile as tile
from concourse import bacc, mybir
from concourse.bass_utils import run_bass_kernel_spmd

NCORES = 8
B = 1024           # batch
M = 4096           # 4^6 kmers
F = 8192           # filters
KMER = 6           # kmer length
NBASE = 4
KK = NBASE * KMER  # 24 flattened (base, position)
FL = F // NCORES   # 1024 filters per core

MT = M // 128      # 32 contraction subtiles of 128
KP = MT // 2       # 16 DoubleRow k-pairs
NBT = B // 128     # 8 batch tiles
SPLIT_BT = 4       # batch tiles covered by the first (overlapped) AllReduce

BF16 = mybir.dt.bfloat16
FP8 = mybir.dt.float8e4
F32 = mybir.dt.float32
AFT = mybir.ActivationFunctionType
ALU = mybir.AluOpType
DR = mybir.MatmulPerfMode.DoubleRow

USE_FP8 = os.environ.get("KERNEL_BF16", "") in ("", "0")

_CACHE: dict = {}


def _body(tc, freqT, oh4, par4, out):
    nc = tc.nc
    stage = os.environ.get("KERNEL_STAGE", "")
    edt = FP8 if USE_FP8 else BF16
    with (
        tc.tile_pool(name="res", bufs=1) as res,
        tc.tile_pool(name="dram", bufs=1, space="DRAM") as dram,
    ):
        # ---------- constants / small inputs ----------
        oh_sb = res.tile([128, 8 * 128], BF16)      # 4-row-packed onehot^T
        nc.sync.dma_start(oh_sb[:], oh4[:])
        par_sb = res.tile([128, FL], BF16)          # 4-row-packed params_eff^T
        nc.sync.dma_start(par_sb[:], par4[:])
        ones_e = res.tile([128, 2, 128], edt)       # DoubleRow ones lhsT
        nc.vector.memset(ones_e[:], 1.0)
        # memset-initialized rhs for PE warm-up matmuls (no DMA dependency)
        warm_rhs = res.tile([128, 2, 512], edt)
        nc.vector.memset(warm_rhs[:], 1.0)

        # ---------- stream in freq^T (fp8/bf16, k-subtile major) ----------
        freq_sb = res.tile([128, MT, B], edt)
        for k in range(MT):
            nc.sync.dma_start(freq_sb[:, k, :], freqT[k * 128:(k + 1) * 128, :])

        E_sb = res.tile([128, MT, FL], edt)
        invz_bc = res.tile([128, FL], F32)
        pooled = res.tile([128, NBT * FL], F32)
        s_col = res.tile([128, NBT], F32)

        # single PSUM pool: pm 2x[128,2,512] + pu 2x[128,1024] = 8 banks
        with tc.tile_pool(name="ps", bufs=1, space="PSUM") as ps:
            zz = ps.tile([128, FL], F32, tag="pu", bufs=2)
            pu0 = ps.tile([128, FL], F32, tag="pu", bufs=2)

            # ---- PE warm-up: dep-free matmuls fill the input-DMA head so
            # the HAM clock gate reaches 8/8 before real compute starts.
            # They target zz with start/stop groups; the real Z accumulation
            # restarts the bank with start=True so the values are discarded.
            for i in range(24):
                if USE_FP8:
                    nc.tensor.matmul(zz[:, 0:512], lhsT=ones_e[:],
                                     rhs=warm_rhs[:], start=True, stop=True,
                                     perf_mode=DR)
                else:
                    nc.tensor.matmul(zz[:, 0:512], lhsT=ones_e[:, 0, :],
                                     rhs=warm_rhs[:, 0, :],
                                     start=True, stop=True)

            def bt_mms(bt, pu, kps, fcs):
                # main-GEMM contributions for batch tile bt, k-pairs kps
                for kp in kps:
                    for fc in fcs:
                        if USE_FP8:
                            nc.tensor.matmul(
                                pu[:, fc * 512:(fc + 1) * 512],
                                lhsT=freq_sb[:, 2 * kp:2 * kp + 2,
                                             bt * 128:(bt + 1) * 128],
                                rhs=E_sb[:, 2 * kp:2 * kp + 2,
                                         fc * 512:(fc + 1) * 512],
                                start=(kp == 0), stop=(kp == KP - 1),
                                perf_mode=DR)
                        else:
                            for k in (2 * kp, 2 * kp + 1):
                                nc.tensor.matmul(
                                    pu[:, fc * 512:(fc + 1) * 512],
                                    lhsT=freq_sb[:, k, bt * 128:(bt + 1) * 128],
                                    rhs=E_sb[:, k, fc * 512:(fc + 1) * 512],
                                    start=(k == 0), stop=(k == MT - 1))

            def z_mms(j, fcs):
                # Z accumulation for the pack produced in iteration j
                for kp, fc in ((kpj, fcj) for kpj in (2 * j, 2 * j + 1)
                               for fcj in fcs):
                    if USE_FP8:
                        nc.tensor.matmul(
                            zz[:, fc * 512:(fc + 1) * 512],
                            lhsT=ones_e[:],
                            rhs=E_sb[:, 2 * kp:2 * kp + 2,
                                     fc * 512:(fc + 1) * 512],
                            start=(kp == 0), stop=(kp == KP - 1),
                            perf_mode=DR)
                    else:
                        for k in (2 * kp, 2 * kp + 1):
                            nc.tensor.matmul(
                                zz[:, fc * 512:(fc + 1) * 512],
                                lhsT=ones_e[:, 0, :],
                                rhs=E_sb[:, k, fc * 512:(fc + 1) * 512],
                                start=(k == 0), stop=(k == MT - 1))

            for j in range(8):          # packs of 4 m-tiles (t = 4j+g)
                for fc in range(2):
                    # plug the early PE-idle hole so the HAM clock gate never
                    # sees a fully-idle window (only legal before the Z
                    # accumulation group opens in iteration (1, 0))
                    if (j, fc) in ((0, 0), (0, 1), (1, 0)):
                        for _ in range(5 if j == 0 else 3):
                            if USE_FP8:
                                nc.tensor.matmul(zz[:, 0:512],
                                                 lhsT=ones_e[:],
                                                 rhs=warm_rhs[:], start=True,
                                                 stop=True, perf_mode=DR)
                            else:
                                nc.tensor.matmul(zz[:, 0:512],
                                                 lhsT=ones_e[:, 0, :],
                                                 rhs=warm_rhs[:, 0, :],
                                                 start=True, stop=True)
                    pmA = ps.tile([128, 2, 512], F32, tag="pm", bufs=2,
                                   name=f"pmA{j}_{fc}")
                    pmB = ps.tile([128, 2, 512], F32, tag="pm", bufs=2,
                                   name=f"pmB{j}_{fc}")
                    for g in range(4):
                        pm = pmA if g < 2 else pmB
                        nc.tensor.matmul(
                            pm[:, g % 2, :],
                            lhsT=oh_sb[32 * g:32 * g + 32,
                                       j * 128:(j + 1) * 128],
                            rhs=par_sb[32 * g:32 * g + 32,
                                       fc * 512:(fc + 1) * 512],
                            start=True, stop=True,
                            tile_position=(32 * g, 0))
                    nc.scalar.activation(
                        E_sb[:, 4 * j:4 * j + 2, fc * 512:(fc + 1) * 512],
                        pmA[:], AFT.Exp)
                    nc.scalar.activation(
                        E_sb[:, 4 * j + 2:4 * j + 4, fc * 512:(fc + 1) * 512],
                        pmB[:], AFT.Exp)
                    if j > 0:
                        z_mms(j - 1, (fc,))
                        bt_mms(0, pu0, (2 * (j - 1), 2 * j - 1), (fc,))
            z_mms(7, (0, 1))
            bt_mms(0, pu0, (14, 15), (0, 1))
            nc.vector.reciprocal(invz_bc[:], zz[:])

            if stage == "1":
                # bisect: dump E k-subtiles bt-shaped
                for bt in range(NBT):
                    prof = res.tile([128, FL], F32, tag="prof1", bufs=2,
                                    name=f"p1_{bt}")
                    nc.scalar.copy(prof[:], E_sb[:, 4 * bt, :])
                    nc.sync.dma_start(out[bt * 128:(bt + 1) * 128, :], prof[:])
                return

            # ------ phase B: U = freq @ E^T; 1/Z scale + rowsum ------
            s_sum = res.tile([128, NBT], F32)
            rinv = res.tile([128, NBT], F32)

            s_in1 = dram.tile([128, SPLIT_BT], F32)
            s_out1 = dram.tile([128, SPLIT_BT], F32, addr_space="Shared")
            s_in2 = dram.tile([128, NBT - SPLIT_BT], F32)
            s_out2 = dram.tile([128, NBT - SPLIT_BT], F32,
                               addr_space="Shared")

            no_cc = bool(os.environ.get("KERNEL_NO_COLLECTIVE"))

            def emit_collective(lo, hi, s_in, s_out):
                if no_cc:
                    nc.vector.tensor_scalar_mul(s_sum[:, lo:hi],
                                                s_col[:, lo:hi],
                                                float(NCORES))
                else:
                    nc.sync.dma_start(s_in[:], s_col[:, lo:hi])
                    nc.gpsimd.collective_compute(
                        "AllReduce", ALU.add,
                        replica_groups=[list(range(NCORES))],
                        ins=[s_in.opt()], outs=[s_out.opt()])
                    nc.sync.dma_start(s_sum[:, lo:hi], s_out[:])
                nc.vector.reciprocal(rinv[:, lo:hi], s_sum[:, lo:hi])

            def bt_epilogue(bt, pu):
                if stage == "2":
                    nc.scalar.copy(pooled[:, bt * FL:(bt + 1) * FL], pu[:])
                    nc.sync.dma_start(out[bt * 128:(bt + 1) * 128, :],
                                      pooled[:, bt * FL:(bt + 1) * FL])
                    return
                nc.vector.tensor_mul(pooled[:, bt * FL:(bt + 1) * FL],
                                     pu[:], invz_bc[:])
                nc.vector.reduce_sum(s_col[:, bt:bt + 1],
                                     pooled[:, bt * FL:(bt + 1) * FL],
                                     axis=mybir.AxisListType.X)
                if bt == SPLIT_BT - 1:
                    emit_collective(0, SPLIT_BT, s_in1, s_out1)

            bt_epilogue(0, pu0)
            for bt in range(1, NBT):
                pu = ps.tile([128, FL], F32, tag="pu", bufs=2,
                             name=f"pu{bt}")
                bt_mms(bt, pu, range(KP), (0, 1))
                bt_epilogue(bt, pu)
            if stage == "2":
                return
            emit_collective(SPLIT_BT, NBT, s_in2, s_out2)

            # ---------- profile = pooled * (1/s); write out ----------
            for bt in range(NBT):
                sl = pooled[:, bt * FL:(bt + 1) * FL]
                if stage == "3":
                    pass  # skip rinv scaling: dump pooled
                elif bt % 2 == 0:
                    nc.scalar.activation(sl, sl, AFT.Copy,
                                         scale=rinv[:, bt:bt + 1])
                else:
                    nc.vector.tensor_scalar_mul(sl, sl, rinv[:, bt:bt + 1])
                nc.sync.dma_start(out[bt * 128:(bt + 1) * 128, :], sl)


def _build_bass():
    nc = bacc.Bacc("TRN2", target_bir_lowering=False, debug=False,
                   num_devices=NCORES)
    idt = FP8 if USE_FP8 else BF16
    freqT = nc.dram_tensor("freqT", [M, B], idt, kind="ExternalInput").ap()
    oh4 = nc.dram_tensor("oh4", [128, 8 * 128], BF16, kind="ExternalInput").ap()
    par4 = nc.dram_tensor("par4", [128, FL], BF16, kind="ExternalInput").ap()
    out = nc.dram_tensor("out", [B, FL], F32, kind="ExternalOutput").ap()

    with tile.TileContext(nc) as tc:
        _body(tc, freqT, oh4, par4, out)
    nc.compile()
    return nc


def _get_nc():
    if "nc" not in _CACHE:
        _CACHE["nc"] = _build_bass()
    return _CACHE["nc"]


def _prepare_in_maps(freq, kmer_params, temperature, kmer_idcs):
    freq = np.asarray(freq, dtype=np.float32)            # (B, M)
    kp = np.asarray(kmer_params, dtype=np.float64)       # (F, 4, K)
    temp = float(np.asarray(temperature, dtype=np.float64).reshape(-1)[0])
    idcs = np.asarray(kmer_idcs).astype(np.int64)        # (M, K)

    assert freq.shape == (B, M) and kp.shape == (F, NBASE, KMER)
    assert idcs.shape == (M, KMER)

    # params_eff folds 1/T, the per-(f, j) max shift (softmax-invariant) and
    # ln(128)/K so that E' = exp(matches_eff) lies in (0, 128].
    shift = kp.max(axis=1) / temp                        # (F, K)
    scale_ln = np.log(128.0) / KMER if USE_FP8 else 0.0
    pf = (kp / temp - shift[:, None, :] + scale_ln)      # (F, 4, K)
    pf_flat = pf.reshape(F, KK).astype(np.float32)       # [f, c*K + j]

    # onehot^T of the kmer index input: ohT[c*K+j, i] = 1 iff idcs[i, j] == c
    onehot = np.zeros((M, NBASE, KMER), dtype=np.float32)
    onehot[np.arange(M)[:, None], idcs, np.arange(KMER)[None, :]] = 1.0
    ohT = onehot.reshape(M, KK).T                        # (24, M)

    # 4-row packing: row group g handles m-tiles t = 4j + g
    oh4 = np.zeros((128, 8, 128), dtype=np.float32)
    for g in range(NBASE):
        for j in range(8):
            t = 4 * j + g
            oh4[32 * g:32 * g + KK, j, :] = ohT[:, t * 128:(t + 1) * 128]
    oh4 = np.ascontiguousarray(
        oh4.reshape(128, 8 * 128)).astype(ml_dtypes.bfloat16)

    if USE_FP8:
        freqT = np.ascontiguousarray(freq.T * 128.0).astype(
            ml_dtypes.float8_e4m3)
    else:
        freqT = np.ascontiguousarray(freq.T).astype(ml_dtypes.bfloat16)

    in_maps = []
    for c in range(NCORES):
        pfc = pf_flat[c * FL:(c + 1) * FL]               # (FL, 24)
        par4 = np.zeros((128, FL), dtype=np.float32)
        for g in range(NBASE):
            par4[32 * g:32 * g + KK, :] = pfc.T
        in_maps.append({
            "freqT": freqT,
            "oh4": oh4,
            "par4": np.ascontiguousarray(par4).astype(ml_dtypes.bfloat16),
        })
    return in_maps


def _run(in_maps, trace=False):
    nc = _get_nc()
    return run_bass_kernel_spmd(nc, in_maps, list(range(NCORES)), trace=trace)


def kernel(freq, kmer_params, temperature, kmer_idcs):
    in_maps = _prepare_in_maps(freq, kmer_params, temperature, kmer_idcs)
    res = _run(in_maps,
               trace=os.environ.get("KERNEL_TRACE", "") not in ("", "0"))
    _CACHE["last_result"] = res
    return np.concatenate(
        [np.asarray(res.results[c]["out"], dtype=np.float32)
         for c in range(NCORES)], axis=1)


# revision 15
# speedup vs baseline: 1.4686x; 1.0555x over previous
"""Trainium2 Bass kernel for the ConvFeatureExtractor problem.

Reference computation (all f32):
    matches[f, i] = sum_j kmer_params[f, kmer_idcs[i, j], j]      # (F, M)
    probs = softmax(matches / temperature, axis=1)                # over M
    pooled = freq @ probs.T                                       # (B, F)
    profile = pooled / pooled.sum(axis=1, keepdims=True)

Shapes: B=1024, M=4096 (=4^6 kmers), F=8192 filters, K=6, 4 bases.

Kernel strategy (8 NeuronCores, filter-sharded: FL = F/8 = 1024 per core):
  * Host folds 1/T, the per-(filter,position) max-shift and a x128 scale
    into params_eff; softmax is invariant to the shift, and the scale
    cancels in the final normalization.  E' = exp(matches_eff) lands in
    (0, 128] which fits fp8e4 (max 240) with all mass in normal range.
  * matches^T via K=24 matmuls, 4-row-packed into the PE array
    (tile_position row groups), exp on ScalarE written as fp8 E.
  * Z[f] = sum_i E[i, f] via DoubleRow ones-matmuls interleaved with
    phase A (broadcast across partitions for free).
  * U = freq @ E^T as fp8 DoubleRow matmuls (2 MACs/cell/cycle).
  * One fused DVE tensor_tensor_reduce per batch tile applies 1/Z and
    produces the per-row sums s in the same pass.
  * s AllReduce over the 8 cores split in two chunks so the first chunk's
    collective latency hides under the tail of the main GEMM.
Each core returns its (B, FL) f32 slice; host concatenates along F.
"""

import os

import numpy as np
import ml_dtypes

import concourse.bass as bass  # noqa: F401
import concourse.tile as tile
from concourse import bacc, mybir
from concourse.bass_utils import run_bass_kernel_spmd

NCORES = 8
B = 1024           # batch
M = 4096           # 4^6 kmers
F = 8192           # filters
KMER = 6           # kmer length
NBASE = 4
KK = NBASE * KMER  # 24 flattened (base, position)
FL = F // NCORES   # 1024 filters per core

MT = M // 128      # 32 contraction subtiles of 128
KP = MT // 2       # 16 DoubleRow k-pairs
NBT = B // 128     # 8 batch tiles
SPLIT_BT = 4       # batch tiles covered by the first (overlapped) AllReduce

BF16 = mybir.dt.bfloat16
FP8 = mybir.dt.float8e4
F32 = mybir.dt.float32
AFT = mybir.ActivationFunctionType
ALU = mybir.AluOpType
DR = mybir.MatmulPerfMode.DoubleRow

USE_FP8 = os.environ.get("KERNEL_BF16", "") in ("", "0")

_CACHE: dict = {}


def _body(tc, freqT, oh4, par4, out):
    nc = tc.nc
    stage = os.environ.get("KERNEL_STAGE", "")
    edt = FP8 if USE_FP8 else BF16
    with (
        tc.tile_pool(name="res", bufs=1) as res,
        tc.tile_pool(name="dram", bufs=1, space="DRAM") as dram,
    ):
        # ---------- constants / small inputs ----------
        oh_sb = res.tile([128, 8 * 128], BF16)      # 4-row-packed onehot^T
        nc.sync.dma_start(oh_sb[:], oh4[:])
        par_sb = res.tile([128, FL], BF16)          # 4-row-packed params_eff^T
        nc.sync.dma_start(par_sb[:], par4[:])
        ones_e = res.tile([128, 2, 128], edt)       # DoubleRow ones lhsT
        nc.vector.memset(ones_e[:], 1.0)
        # memset-initialized rhs for PE warm-up matmuls (no DMA dependency)
        warm_rhs = res.tile([128, 2, 512], edt)
        nc.vector.memset(warm_rhs[:], 1.0)

        # ---------- stream in freq^T (fp8/bf16, k-subtile major) ----------
        freq_sb = res.tile([128, MT, B], edt)
        for k in range(MT):
            nc.sync.dma_start(freq_sb[:, k, :], freqT[k * 128:(k + 1) * 128, :])

        E_sb = res.tile([128, MT, FL], edt)
        invz_bc = res.tile([128, FL], F32)
        pooled = res.tile([128, NBT * FL], F32)
        s_col = res.tile([128, NBT], F32)

        # single PSUM pool: pm 2x[128,2,512] + pu 2x[128,1024] = 8 banks
        with tc.tile_pool(name="ps", bufs=1, space="PSUM") as ps:
            zz = ps.tile([128, FL], F32, tag="pu", bufs=2)
            pu0 = ps.tile([128, FL], F32, tag="pu", bufs=2)

            # ---- PE warm-up: dep-free matmuls fill the input-DMA head so
            # the HAM clock gate reaches 8/8 before real compute starts.
            # They target zz with start/stop groups; the real Z accumulation
            # restarts the bank with start=True so the values are discarded.
            for i in range(12):
                if USE_FP8:
                    nc.tensor.matmul(zz[:, 0:512], lhsT=ones_e[:],
                                     rhs=warm_rhs[:], start=True, stop=True,
                                     perf_mode=DR)
                else:
                    nc.tensor.matmul(zz[:, 0:512], lhsT=ones_e[:, 0, :],
                                     rhs=warm_rhs[:, 0, :],
                                     start=True, stop=True)

            def bt_mms(bt, pu, kps, fcs):
                # main-GEMM contributions for batch tile bt, k-pairs kps
                for kp in kps:
                    for fc in fcs:
                        if USE_FP8:
                            nc.tensor.matmul(
                                pu[:, fc * 512:(fc + 1) * 512],
                                lhsT=freq_sb[:, 2 * kp:2 * kp + 2,
                                             bt * 128:(bt + 1) * 128],
                                rhs=E_sb[:, 2 * kp:2 * kp + 2,
                                         fc * 512:(fc + 1) * 512],
                                start=(kp == 0), stop=(kp == KP - 1),
                                perf_mode=DR)
                        else:
                            for k in (2 * kp, 2 * kp + 1):
                                nc.tensor.matmul(
                                    pu[:, fc * 512:(fc + 1) * 512],
                                    lhsT=freq_sb[:, k, bt * 128:(bt + 1) * 128],
                                    rhs=E_sb[:, k, fc * 512:(fc + 1) * 512],
                                    start=(k == 0), stop=(k == MT - 1))

            def z_mms(j, fcs):
                # Z accumulation for the pack produced in iteration j
                for kp, fc in ((kpj, fcj) for kpj in (2 * j, 2 * j + 1)
                               for fcj in fcs):
                    if USE_FP8:
                        nc.tensor.matmul(
                            zz[:, fc * 512:(fc + 1) * 512],
                            lhsT=ones_e[:],
                            rhs=E_sb[:, 2 * kp:2 * kp + 2,
                                     fc * 512:(fc + 1) * 512],
                            start=(kp == 0), stop=(kp == KP - 1),
                            perf_mode=DR)
                    else:
                        for k in (2 * kp, 2 * kp + 1):
                            nc.tensor.matmul(
                                zz[:, fc * 512:(fc + 1) * 512],
                                lhsT=ones_e[:, 0, :],
                                rhs=E_sb[:, k, fc * 512:(fc + 1) * 512],
                                start=(k == 0), stop=(k == MT - 1))

            for j in range(8):          # packs of 4 m-tiles (t = 4j+g)
                for fc in range(2):
                    # plug the early PE-idle hole so the HAM clock gate never
                    # sees a fully-idle window (only legal before the Z
                    # accumulation group opens in iteration (1, 0))
                    if (j, fc) in ((0, 0), (0, 1), (1, 0)):
                        for _ in range(5 if j == 0 else 3):
                            if USE_FP8:
                                nc.tensor.matmul(zz[:, 0:512],
                                                 lhsT=ones_e[:],
                                                 rhs=warm_rhs[:], start=True,
                                                 stop=True, perf_mode=DR)
                            else:
                                nc.tensor.matmul(zz[:, 0:512],
                                                 lhsT=ones_e[:, 0, :],
                                                 rhs=warm_rhs[:, 0, :],
                                                 start=True, stop=True)
                    pmA = ps.tile([128, 2, 512], F32, tag="pm", bufs=2,
                                   name=f"pmA{j}_{fc}")
                    pmB = ps.tile([128, 2, 512], F32, tag="pm", bufs=2,
                                   name=f"pmB{j}_{fc}")
                    for g in range(4):
                        pm = pmA if g < 2 else pmB
                        nc.tensor.matmul(
                            pm[:, g % 2, :],
                            lhsT=oh_sb[32 * g:32 * g + 32,
                                       j * 128:(j + 1) * 128],
                            rhs=par_sb[32 * g:32 * g + 32,
                                       fc * 512:(fc + 1) * 512],
                            start=True, stop=True,
                            tile_position=(32 * g, 0))
                    nc.scalar.activation(
                        E_sb[:, 4 * j:4 * j + 2, fc * 512:(fc + 1) * 512],
                        pmA[:], AFT.Exp)
                    nc.scalar.activation(
                        E_sb[:, 4 * j + 2:4 * j + 4, fc * 512:(fc + 1) * 512],
                        pmB[:], AFT.Exp)
                    if j > 0:
                        z_mms(j - 1, (fc,))
                        bt_mms(0, pu0, (2 * (j - 1), 2 * j - 1), (fc,))
            z_mms(7, (0, 1))
            bt_mms(0, pu0, (14, 15), (0, 1))
            # seam fill: keep the PE busy (and the HAM gate warm) while the
            # Z reciprocal frees the pu bank that batch tile 1 will reuse
            for _ in range(14):
                if USE_FP8:
                    nc.tensor.matmul(pmA[:, 0, :], lhsT=ones_e[:],
                                     rhs=warm_rhs[:], start=True, stop=True,
                                     perf_mode=DR)
                else:
                    nc.tensor.matmul(pmA[:, 0, :], lhsT=ones_e[:, 0, :],
                                     rhs=warm_rhs[:, 0, :],
                                     start=True, stop=True)
            nc.vector.reciprocal(invz_bc[:], zz[:])

            if stage == "1":
                # bisect: dump E k-subtiles bt-shaped
                for bt in range(NBT):
                    prof = res.tile([128, FL], F32, tag="prof1", bufs=2,
                                    name=f"p1_{bt}")
                    nc.scalar.copy(prof[:], E_sb[:, 4 * bt, :])
                    nc.sync.dma_start(out[bt * 128:(bt + 1) * 128, :], prof[:])
                return

            # ------ phase B: U = freq @ E^T; 1/Z scale + rowsum ------
            s_sum = res.tile([128, NBT], F32)
            rinv = res.tile([128, NBT], F32)

            s_in1 = dram.tile([128, SPLIT_BT], F32)
            s_out1 = dram.tile([128, SPLIT_BT], F32, addr_space="Shared")
            s_in2 = dram.tile([128, NBT - SPLIT_BT], F32)
            s_out2 = dram.tile([128, NBT - SPLIT_BT], F32,
                               addr_space="Shared")

            no_cc = bool(os.environ.get("KERNEL_NO_COLLECTIVE"))

            def emit_collective(lo, hi, s_in, s_out):
                if no_cc:
                    nc.vector.tensor_scalar_mul(s_sum[:, lo:hi],
                                                s_col[:, lo:hi],
                                                float(NCORES))
                else:
                    nc.sync.dma_start(s_in[:], s_col[:, lo:hi])
                    nc.gpsimd.collective_compute(
                        "AllReduce", ALU.add,
                        replica_groups=[list(range(NCORES))],
                        ins=[s_in.opt()], outs=[s_out.opt()])
                    nc.sync.dma_start(s_sum[:, lo:hi], s_out[:])
                nc.vector.reciprocal(rinv[:, lo:hi], s_sum[:, lo:hi])

            def bt_epilogue(bt, pu):
                if stage == "2":
                    nc.scalar.copy(pooled[:, bt * FL:(bt + 1) * FL], pu[:])
                    nc.sync.dma_start(out[bt * 128:(bt + 1) * 128, :],
                                      pooled[:, bt * FL:(bt + 1) * FL])
                    return
                nc.vector.tensor_mul(pooled[:, bt * FL:(bt + 1) * FL],
                                     pu[:], invz_bc[:])
                nc.vector.reduce_sum(s_col[:, bt:bt + 1],
                                     pooled[:, bt * FL:(bt + 1) * FL],
                                     axis=mybir.AxisListType.X)
                if bt == SPLIT_BT - 1:
                    emit_collective(0, SPLIT_BT, s_in1, s_out1)

            bt_epilogue(0, pu0)
            for bt in range(1, NBT):
                pu = ps.tile([128, FL], F32, tag="pu", bufs=2,
                             name=f"pu{bt}")
                bt_mms(bt, pu, range(KP), (0, 1))
                bt_epilogue(bt, pu)
            if stage == "2":
                return
            emit_collective(SPLIT_BT, NBT, s_in2, s_out2)

            # ---------- profile = pooled * (1/s); write out ----------
            for bt in range(NBT):
                sl = pooled[:, bt * FL:(bt + 1) * FL]
                if stage == "3":
                    pass  # skip rinv scaling: dump pooled
                elif bt % 2 == 0:
                    nc.scalar.activation(sl, sl, AFT.Copy,
                                         scale=rinv[:, bt:bt + 1])
                else:
                    nc.vector.tensor_scalar_mul(sl, sl, rinv[:, bt:bt + 1])
                nc.sync.dma_start(out[bt * 128:(bt + 1) * 128, :], sl)


def _build_bass():
    nc = bacc.Bacc("TRN2", target_bir_lowering=False, debug=False,
                   num_devices=NCORES)
    idt = FP8 if USE_FP8 else BF16
    freqT = nc.dram_tensor("freqT", [M, B], idt, kind="ExternalInput").ap()
    oh4 = nc.dram_tensor("oh4", [128, 8 * 128], BF16, kind="ExternalInput").ap()
    par4 = nc.dram_tensor("par4", [128, FL], BF16, kind="ExternalInput").ap()
    out = nc.dram_tensor("out", [B, FL], F32, kind="ExternalOutput").ap()

    with tile.TileContext(nc) as tc:
        _body(tc, freqT, oh4, par4, out)
    nc.compile()
    return nc


def _get_nc():
    if "nc" not in _CACHE:
        _CACHE["nc"] = _build_bass()
    return _CACHE["nc"]


def _prepare_in_maps(freq, kmer_params, temperature, kmer_idcs):
    freq = np.asarray(freq, dtype=np.float32)            # (B, M)
    kp = np.asarray(kmer_params, dtype=np.float64)       # (F, 4, K)
    temp = float(np.asarray(temperature, dtype=np.float64).reshape(-1)[0])
    idcs = np.asarray(kmer_idcs).astype(np.int64)        # (M, K)

    assert freq.shape == (B, M) and kp.shape == (F, NBASE, KMER)
    assert idcs.shape == (M, KMER)

    # params_eff folds 1/T, the per-(f, j) max shift (softmax-invariant) and
    # ln(128)/K so that E' = exp(matches_eff) lies in (0, 128].
    shift = kp.max(axis=1) / temp                        # (F, K)
    scale_ln = np.log(128.0) / KMER if USE_FP8 else 0.0
    pf = (kp / temp - shift[:, None, :] + scale_ln)      # (F, 4, K)
    pf_flat = pf.reshape(F, KK).astype(np.float32)       # [f, c*K + j]

    # onehot^T of the kmer index input: ohT[c*K+j, i] = 1 iff idcs[i, j] == c
    onehot = np.zeros((M, NBASE, KMER), dtype=np.float32)
    onehot[np.arange(M)[:, None], idcs, np.arange(KMER)[None, :]] = 1.0
    ohT = onehot.reshape(M, KK).T                        # (24, M)

    # 4-row packing: row group g handles m-tiles t = 4j + g
    oh4 = np.zeros((128, 8, 128), dtype=np.float32)
    for g in range(NBASE):
        for j in range(8):
            t = 4 * j + g
            oh4[32 * g:32 * g + KK, j, :] = ohT[:, t * 128:(t + 1) * 128]
    oh4 = np.ascontiguousarray(
        oh4.reshape(128, 8 * 128)).astype(ml_dtypes.bfloat16)

    if USE_FP8:
        freqT = np.ascontiguousarray(freq.T * 128.0).astype(
            ml_dtypes.float8_e4m3)
    else:
        freqT = np.ascontiguousarray(freq.T).astype(ml_dtypes.bfloat16)

    in_maps = []
    for c in range(NCORES):
        pfc = pf_flat[c * FL:(c + 1) * FL]               # (FL, 24)
        par4 = np.zeros((128, FL), dtype=np.float32)
        for g in range(NBASE):
            par4[32 * g:32 * g + KK, :] = pfc.T
        in_maps.append({
            "freqT": freqT,
            "oh4": oh4,
            "par4": np.ascontiguousarray(par4).astype(ml_dtypes.bfloat16),
        })
    return in_maps


def _run(in_maps, trace=False):
    nc = _get_nc()
    return run_bass_kernel_spmd(nc, in_maps, list(range(NCORES)), trace=trace)


def kernel(freq, kmer_params, temperature, kmer_idcs):
    in_maps = _prepare_in_maps(freq, kmer_params, temperature, kmer_idcs)
    res = _run(in_maps,
               trace=os.environ.get("KERNEL_TRACE", "") not in ("", "0"))
    _CACHE["last_result"] = res
    return np.concatenate(
        [np.asarray(res.results[c]["out"], dtype=np.float32)
         for c in range(NCORES)], axis=1)


# revision 16
# speedup vs baseline: 1.4779x; 1.0063x over previous
"""Trainium2 Bass kernel for the ConvFeatureExtractor problem.

Reference computation (all f32):
    matches[f, i] = sum_j kmer_params[f, kmer_idcs[i, j], j]      # (F, M)
    probs = softmax(matches / temperature, axis=1)                # over M
    pooled = freq @ probs.T                                       # (B, F)
    profile = pooled / pooled.sum(axis=1, keepdims=True)

Shapes: B=1024, M=4096 (=4^6 kmers), F=8192 filters, K=6, 4 bases.

Kernel strategy (8 NeuronCores, filter-sharded: FL = F/8 = 1024 per core):
  * Host folds 1/T, the per-(filter,position) max-shift and a x128 scale
    into params_eff; softmax is invariant to the shift, and the scale
    cancels in the final normalization.  E' = exp(matches_eff) lands in
    (0, 128] which fits fp8e4 (max 240) with all mass in normal range.
  * matches^T via K=24 matmuls, 4-row-packed into the PE array
    (tile_position row groups), exp on ScalarE written as fp8 E.
  * Z[f] = sum_i E[i, f] via DoubleRow ones-matmuls interleaved with
    phase A (broadcast across partitions for free).
  * U = freq @ E^T as fp8 DoubleRow matmuls (2 MACs/cell/cycle).
  * One fused DVE tensor_tensor_reduce per batch tile applies 1/Z and
    produces the per-row sums s in the same pass.
  * s AllReduce over the 8 cores split in two chunks so the first chunk's
    collective latency hides under the tail of the main GEMM.
Each core returns its (B, FL) f32 slice; host concatenates along F.
"""

import os

import numpy as np
import ml_dtypes

import concourse.bass as bass  # noqa: F401
import concourse.tile as tile
from concourse import bacc, mybir
from concourse.bass_utils import run_bass_kernel_spmd

NCORES = 8
B = 1024           # batch
M = 4096           # 4^6 kmers
F = 8192           # filters
KMER = 6           # kmer length
NBASE = 4
KK = NBASE * KMER  # 24 flattened (base, position)
FL = F // NCORES   # 1024 filters per core

MT = M // 128      # 32 contraction subtiles of 128
KP = MT // 2       # 16 DoubleRow k-pairs
NBT = B // 128     # 8 batch tiles
SPLIT_BT = 4       # batch tiles covered by the first (overlapped) AllReduce

BF16 = mybir.dt.bfloat16
FP8 = mybir.dt.float8e4
F32 = mybir.dt.float32
AFT = mybir.ActivationFunctionType
ALU = mybir.AluOpType
DR = mybir.MatmulPerfMode.DoubleRow

USE_FP8 = os.environ.get("KERNEL_BF16", "") in ("", "0")

_CACHE: dict = {}


def _body(tc, freqT, oh4, par4, out):
    nc = tc.nc
    stage = os.environ.get("KERNEL_STAGE", "")
    edt = FP8 if USE_FP8 else BF16
    with (
        tc.tile_pool(name="res", bufs=1) as res,
        tc.tile_pool(name="dram", bufs=1, space="DRAM") as dram,
    ):
        # ---------- constants / small inputs ----------
        oh_sb = res.tile([128, 8 * 128], BF16)      # 4-row-packed onehot^T
        nc.sync.dma_start(oh_sb[:], oh4[:])
        par_sb = res.tile([128, FL], BF16)          # 4-row-packed params_eff^T
        nc.sync.dma_start(par_sb[:], par4[:])
        ones_e = res.tile([128, 2, 128], edt)       # DoubleRow ones lhsT
        nc.vector.memset(ones_e[:], 1.0)
        # memset-initialized rhs for PE warm-up matmuls (no DMA dependency)
        warm_rhs = res.tile([128, 2, 512], edt)
        nc.vector.memset(warm_rhs[:], 1.0)

        # ---------- stream in freq^T (fp8/bf16, k-subtile major) ----------
        freq_sb = res.tile([128, MT, B], edt)
        for k in range(MT):
            nc.sync.dma_start(freq_sb[:, k, :], freqT[k * 128:(k + 1) * 128, :])

        E_sb = res.tile([128, MT, FL], edt)
        invz_bc = res.tile([128, FL], F32)
        pooled = res.tile([128, NBT * FL], F32)
        s_col = res.tile([128, NBT], F32)

        # single PSUM pool: pm 2x[128,2,512] + pu 2x[128,1024] = 8 banks
        with tc.tile_pool(name="ps", bufs=1, space="PSUM") as ps:
            zz = ps.tile([128, FL], F32, tag="pu", bufs=2)
            pu0 = ps.tile([128, FL], F32, tag="pu", bufs=2)

            # ---- PE warm-up: dep-free matmuls fill the input-DMA head so
            # the HAM clock gate reaches 8/8 before real compute starts.
            # They target zz with start/stop groups; the real Z accumulation
            # restarts the bank with start=True so the values are discarded.
            for i in range(12):
                if USE_FP8:
                    nc.tensor.matmul(zz[:, 0:512], lhsT=ones_e[:],
                                     rhs=warm_rhs[:], start=True, stop=True,
                                     perf_mode=DR)
                else:
                    nc.tensor.matmul(zz[:, 0:512], lhsT=ones_e[:, 0, :],
                                     rhs=warm_rhs[:, 0, :],
                                     start=True, stop=True)

            def bt_mms(bt, pu, kps, fcs):
                # main-GEMM contributions for batch tile bt, k-pairs kps
                for kp in kps:
                    for fc in fcs:
                        if USE_FP8:
                            nc.tensor.matmul(
                                pu[:, fc * 512:(fc + 1) * 512],
                                lhsT=freq_sb[:, 2 * kp:2 * kp + 2,
                                             bt * 128:(bt + 1) * 128],
                                rhs=E_sb[:, 2 * kp:2 * kp + 2,
                                         fc * 512:(fc + 1) * 512],
                                start=(kp == 0), stop=(kp == KP - 1),
                                perf_mode=DR)
                        else:
                            for k in (2 * kp, 2 * kp + 1):
                                nc.tensor.matmul(
                                    pu[:, fc * 512:(fc + 1) * 512],
                                    lhsT=freq_sb[:, k, bt * 128:(bt + 1) * 128],
                                    rhs=E_sb[:, k, fc * 512:(fc + 1) * 512],
                                    start=(k == 0), stop=(k == MT - 1))

            def z_mms(j, fcs):
                # Z accumulation for the pack produced in iteration j
                for kp, fc in ((kpj, fcj) for kpj in (2 * j, 2 * j + 1)
                               for fcj in fcs):
                    if USE_FP8:
                        nc.tensor.matmul(
                            zz[:, fc * 512:(fc + 1) * 512],
                            lhsT=ones_e[:],
                            rhs=E_sb[:, 2 * kp:2 * kp + 2,
                                     fc * 512:(fc + 1) * 512],
                            start=(kp == 0), stop=(kp == KP - 1),
                            perf_mode=DR)
                    else:
                        for k in (2 * kp, 2 * kp + 1):
                            nc.tensor.matmul(
                                zz[:, fc * 512:(fc + 1) * 512],
                                lhsT=ones_e[:, 0, :],
                                rhs=E_sb[:, k, fc * 512:(fc + 1) * 512],
                                start=(k == 0), stop=(k == MT - 1))

            for j in range(8):          # packs of 4 m-tiles (t = 4j+g)
                for fc in range(2):
                    # plug the early PE-idle hole so the HAM clock gate never
                    # sees a fully-idle window (only legal before the Z
                    # accumulation group opens in iteration (1, 0))
                    if (j, fc) in ((0, 0), (0, 1), (1, 0)):
                        for _ in range(5 if j == 0 else 3):
                            if USE_FP8:
                                nc.tensor.matmul(zz[:, 0:512],
                                                 lhsT=ones_e[:],
                                                 rhs=warm_rhs[:], start=True,
                                                 stop=True, perf_mode=DR)
                            else:
                                nc.tensor.matmul(zz[:, 0:512],
                                                 lhsT=ones_e[:, 0, :],
                                                 rhs=warm_rhs[:, 0, :],
                                                 start=True, stop=True)
                    pmA = ps.tile([128, 2, 512], F32, tag="pm", bufs=2,
                                   name=f"pmA{j}_{fc}")
                    pmB = ps.tile([128, 2, 512], F32, tag="pm", bufs=2,
                                   name=f"pmB{j}_{fc}")
                    for g in range(4):
                        pm = pmA if g < 2 else pmB
                        nc.tensor.matmul(
                            pm[:, g % 2, :],
                            lhsT=oh_sb[32 * g:32 * g + 32,
                                       j * 128:(j + 1) * 128],
                            rhs=par_sb[32 * g:32 * g + 32,
                                       fc * 512:(fc + 1) * 512],
                            start=True, stop=True,
                            tile_position=(32 * g, 0))
                    nc.scalar.activation(
                        E_sb[:, 4 * j:4 * j + 2, fc * 512:(fc + 1) * 512],
                        pmA[:], AFT.Exp)
                    nc.scalar.activation(
                        E_sb[:, 4 * j + 2:4 * j + 4, fc * 512:(fc + 1) * 512],
                        pmB[:], AFT.Exp)
                    if j > 0:
                        z_mms(j - 1, (fc,))
                        bt_mms(0, pu0, (2 * (j - 1), 2 * j - 1), (fc,))
            z_mms(7, (0, 1))
            bt_mms(0, pu0, (14, 15), (0, 1))
            # seam fill: keep the PE busy (and the HAM gate warm) while the
            # Z reciprocal frees the pu bank that batch tile 1 will reuse
            for _ in range(28):
                if USE_FP8:
                    nc.tensor.matmul(pmA[:, 0, :], lhsT=ones_e[:],
                                     rhs=warm_rhs[:], start=True, stop=True,
                                     perf_mode=DR)
                else:
                    nc.tensor.matmul(pmA[:, 0, :], lhsT=ones_e[:, 0, :],
                                     rhs=warm_rhs[:, 0, :],
                                     start=True, stop=True)
            nc.vector.reciprocal(invz_bc[:, 0:512], zz[:, 0:512])
            nc.vector.reciprocal(invz_bc[:, 512:1024], zz[:, 512:1024])

            if stage == "1":
                # bisect: dump E k-subtiles bt-shaped
                for bt in range(NBT):
                    prof = res.tile([128, FL], F32, tag="prof1", bufs=2,
                                    name=f"p1_{bt}")
                    nc.scalar.copy(prof[:], E_sb[:, 4 * bt, :])
                    nc.sync.dma_start(out[bt * 128:(bt + 1) * 128, :], prof[:])
                return

            # ------ phase B: U = freq @ E^T; 1/Z scale + rowsum ------
            s_sum = res.tile([128, NBT], F32)
            rinv = res.tile([128, NBT], F32)

            s_in1 = dram.tile([128, SPLIT_BT], F32)
            s_out1 = dram.tile([128, SPLIT_BT], F32, addr_space="Shared")
            s_in2 = dram.tile([128, NBT - SPLIT_BT], F32)
            s_out2 = dram.tile([128, NBT - SPLIT_BT], F32,
                               addr_space="Shared")

            no_cc = bool(os.environ.get("KERNEL_NO_COLLECTIVE"))

            def emit_collective(lo, hi, s_in, s_out):
                if no_cc:
                    nc.vector.tensor_scalar_mul(s_sum[:, lo:hi],
                                                s_col[:, lo:hi],
                                                float(NCORES))
                else:
                    nc.sync.dma_start(s_in[:], s_col[:, lo:hi])
                    nc.gpsimd.collective_compute(
                        "AllReduce", ALU.add,
                        replica_groups=[list(range(NCORES))],
                        ins=[s_in.opt()], outs=[s_out.opt()])
                    nc.sync.dma_start(s_sum[:, lo:hi], s_out[:])
                nc.vector.reciprocal(rinv[:, lo:hi], s_sum[:, lo:hi])

            def bt_epilogue(bt, pu):
                if stage == "2":
                    nc.scalar.copy(pooled[:, bt * FL:(bt + 1) * FL], pu[:])
                    nc.sync.dma_start(out[bt * 128:(bt + 1) * 128, :],
                                      pooled[:, bt * FL:(bt + 1) * FL])
                    return
                nc.vector.tensor_mul(pooled[:, bt * FL:(bt + 1) * FL],
                                     pu[:], invz_bc[:])
                nc.vector.reduce_sum(s_col[:, bt:bt + 1],
                                     pooled[:, bt * FL:(bt + 1) * FL],
                                     axis=mybir.AxisListType.X)
                if bt == SPLIT_BT - 1:
                    emit_collective(0, SPLIT_BT, s_in1, s_out1)

            bt_epilogue(0, pu0)
            for bt in range(1, NBT):
                pu = ps.tile([128, FL], F32, tag="pu", bufs=2,
                             name=f"pu{bt}")
                bt_mms(bt, pu, range(KP), (0, 1))
                bt_epilogue(bt, pu)
            if stage == "2":
                return
            emit_collective(SPLIT_BT, NBT, s_in2, s_out2)

            # ---------- profile = pooled * (1/s); write out ----------
            for bt in range(NBT):
                sl = pooled[:, bt * FL:(bt + 1) * FL]
                if stage == "3":
                    pass  # skip rinv scaling: dump pooled
                elif bt % 2 == 0:
                    nc.scalar.activation(sl, sl, AFT.Copy,
                                         scale=rinv[:, bt:bt + 1])
                else:
                    nc.vector.tensor_scalar_mul(sl, sl, rinv[:, bt:bt + 1])
                nc.sync.dma_start(out[bt * 128:(bt + 1) * 128, :], sl)


def _build_bass():
    nc = bacc.Bacc("TRN2", target_bir_lowering=False, debug=False,
                   num_devices=NCORES)
    idt = FP8 if USE_FP8 else BF16
    freqT = nc.dram_tensor("freqT", [M, B], idt, kind="ExternalInput").ap()
    oh4 = nc.dram_tensor("oh4", [128, 8 * 128], BF16, kind="ExternalInput").ap()
    par4 = nc.dram_tensor("par4", [128, FL], BF16, kind="ExternalInput").ap()
    out = nc.dram_tensor("out", [B, FL], F32, kind="ExternalOutput").ap()

    with tile.TileContext(nc) as tc:
        _body(tc, freqT, oh4, par4, out)
    nc.compile()
    return nc


def _get_nc():
    if "nc" not in _CACHE:
        _CACHE["nc"] = _build_bass()
    return _CACHE["nc"]


def _prepare_in_maps(freq, kmer_params, temperature, kmer_idcs):
    freq = np.asarray(freq, dtype=np.float32)            # (B, M)
    kp = np.asarray(kmer_params, dtype=np.float64)       # (F, 4, K)
    temp = float(np.asarray(temperature, dtype=np.float64).reshape(-1)[0])
    idcs = np.asarray(kmer_idcs).astype(np.int64)        # (M, K)

    assert freq.shape == (B, M) and kp.shape == (F, NBASE, KMER)
    assert idcs.shape == (M, KMER)

    # params_eff folds 1/T, the per-(f, j) max shift (softmax-invariant) and
    # ln(128)/K so that E' = exp(matches_eff) lies in (0, 128].
    shift = kp.max(axis=1) / temp                        # (F, K)
    scale_ln = np.log(128.0) / KMER if USE_FP8 else 0.0
    pf = (kp / temp - shift[:, None, :] + scale_ln)      # (F, 4, K)
    pf_flat = pf.reshape(F, KK).astype(np.float32)       # [f, c*K + j]

    # onehot^T of the kmer index input: ohT[c*K+j, i] = 1 iff idcs[i, j] == c
    onehot = np.zeros((M, NBASE, KMER), dtype=np.float32)
    onehot[np.arange(M)[:, None], idcs, np.arange(KMER)[None, :]] = 1.0
    ohT = onehot.reshape(M, KK).T                        # (24, M)

    # 4-row packing: row group g handles m-tiles t = 4j + g
    oh4 = np.zeros((128, 8, 128), dtype=np.float32)
    for g in range(NBASE):
        for j in range(8):
            t = 4 * j + g
            oh4[32 * g:32 * g + KK, j, :] = ohT[:, t * 128:(t + 1) * 128]
    oh4 = np.ascontiguousarray(
        oh4.reshape(128, 8 * 128)).astype(ml_dtypes.bfloat16)

    if USE_FP8:
        freqT = np.ascontiguousarray(freq.T * 128.0).astype(
            ml_dtypes.float8_e4m3)
    else:
        freqT = np.ascontiguousarray(freq.T).astype(ml_dtypes.bfloat16)

    in_maps = []
    for c in range(NCORES):
        pfc = pf_flat[c * FL:(c + 1) * FL]               # (FL, 24)
        par4 = np.zeros((128, FL), dtype=np.float32)
        for g in range(NBASE):
            par4[32 * g:32 * g + KK, :] = pfc.T
        in_maps.append({
            "freqT": freqT,
            "oh4": oh4,
            "par4": np.ascontiguousarray(par4).astype(ml_dtypes.bfloat16),
        })
    return in_maps


def _run(in_maps, trace=False):
    nc = _get_nc()
    return run_bass_kernel_spmd(nc, in_maps, list(range(NCORES)), trace=trace)


def kernel(freq, kmer_params, temperature, kmer_idcs):
    in_maps = _prepare_in_maps(freq, kmer_params, temperature, kmer_idcs)
    res = _run(in_maps,
               trace=os.environ.get("KERNEL_TRACE", "") not in ("", "0"))
    _CACHE["last_result"] = res
    return np.concatenate(
        [np.asarray(res.results[c]["out"], dtype=np.float32)
         for c in range(NCORES)], axis=1)
